# revision 1
# baseline (speedup 1.0000x reference)
"""TRN2 Bass kernel for nn_CrossLayerAttention: head-parallel tensor-parallel
over 8 NeuronCores.

Decomposition per core i (2 heads, local channel slice sl = [256i, 256i+256)):
  - hT0/hT1/hT2 = h.T, host pre-transposed and cast to bf16, streamed from DRAM
  - QT_h = R2*diag(qn)*Wq[sl] @ h2.T (rope+qn folded into weights on host;
    rmsnorm scale computed from the roped output, valid because rope is
    orthogonal when qn==1)
  - KT similarly for h0 (depth 0) and h1 (depth 1); V projected
    weight-stationary as VT then PE-transposed to natural layout
  - attention in ST layout: E = exp(KTn.T @ QTn / sqrt(D)); Z accumulated in
    fp32 on DVE + one fp32r ones-matmul broadcast; OT = V.T @ E * (1/Z)
  - out_proj + SIREN positional field accumulated into a per-core partial;
    attention runs in two q-block pairs so out_proj + chunked ReduceScatter
    overlap the second pair; final rmsnorm + residual on the shard in fp32
Matmuls run in bf16 (fp32 PSUM accumulation); softmax normalization, rmsnorm
chains and the residual epilogue stay fp32.
"""
import numpy as np
import ml_dtypes
from contextlib import ExitStack

import concourse.bass as bass
import concourse.tile as tile
from concourse import bacc, mybir
from concourse.bass_utils import run_bass_kernel_spmd

P = 128
L = 2048
C = 2048
H = 16
D = 128
NCORES = 8
HPC = H // NCORES          # heads per core
CL = HPC * D               # local channels per core
LKV = 2 * L                # kv length (2 history entries)
EPS = 1e-6
NQB = L // 512             # q blocks / RS chunks (4)
NCK = LKV // P             # kv chunks (32)
NCC = C // P               # contraction chunks (16)
SH = L // 8 // NQB         # shard rows per RS chunk (64)

f32 = mybir.dt.float32
f32r = mybir.dt.float32r
bf16 = mybir.dt.bfloat16
i32 = mybir.dt.int32
FT = mybir.ActivationFunctionType
OP = mybir.AluOpType
BF = ml_dtypes.bfloat16

_CACHE = {}


def _build_program():
    nc = bacc.Bacc("TRN2", target_bir_lowering=False, debug=False,
                   num_devices=NCORES)

    # ---- DRAM I/O ----
    hT = [nc.dram_tensor(f"hT{t}", [C, L], bf16, kind="ExternalInput")
          for t in range(3)]
    wq = nc.dram_tensor("wq", [C, CL], bf16, kind="ExternalInput")
    wk0 = nc.dram_tensor("wk0", [C, CL], bf16, kind="ExternalInput")
    wk1 = nc.dram_tensor("wk1", [C, CL], bf16, kind="ExternalInput")
    wv = nc.dram_tensor("wv", [C, CL], bf16, kind="ExternalInput")
    wo = nc.dram_tensor("wo", [CL, C], bf16, kind="ExternalInput")
    sw2l = nc.dram_tensor("sw2l", [CL, C], bf16, kind="ExternalInput")
    coef = nc.dram_tensor("coef", [P, 6], f32, kind="ExternalInput")
    ident = nc.dram_tensor("ident", [P, P], f32, kind="ExternalInput")
    onw = nc.dram_tensor("onw", [P, C], f32, kind="ExternalInput")
    xs = nc.dram_tensor("xs", [NQB * SH, C], f32, kind="ExternalInput")
    out = nc.dram_tensor("o", [NQB * SH, C], f32, kind="ExternalOutput")

    partial = [nc.dram_tensor(f"partial{k}", [512, C], f32) for k in range(NQB)]
    rs_out = [nc.dram_tensor(f"rs_out{k}", [SH, C], f32) for k in range(NQB)]

    with tile.TileContext(nc) as tc, ExitStack() as ctx:
        const = ctx.enter_context(tc.tile_pool(name="const", bufs=1))
        persist = ctx.enter_context(tc.tile_pool(name="persist", bufs=1))

        # ---- constants ----
        ones_t = const.tile([P, P], f32)
        nc.vector.memset(ones_t[:], 1.0)
        ones_b = const.tile([P, P], bf16)
        nc.vector.tensor_copy(ones_b[:], ones_t[:])
        ones_r = const.tile([P, P], f32)
        nc.vector.tensor_copy(ones_r[:].bitcast(f32r), ones_t[:])
        onesr = ones_r[:].bitcast(f32r)
        coef_sb = const.tile([P, 6], f32)
        nc.sync.dma_start(coef_sb[:], coef[:])
        ident_sb = const.tile([P, P], f32)
        nc.sync.dma_start(ident_sb[:], ident[:])

        # ---- persistent across attention / out_proj (bf16) ----
        OTn = [persist.tile([P, L], bf16, name=f"OTn{h}") for h in range(HPC)]
        sinT = [persist.tile([P, L], bf16, name=f"sinT{j}") for j in range(2)]

        acts_cm = tc.tile_pool(name="acts", bufs=1)
        acts = acts_cm.__enter__()
        misc_cm = tc.tile_pool(name="misc", bufs=3)
        misc = misc_cm.__enter__()
        QTn = [acts.tile([P, L], bf16, name=f"QTn{h}") for h in range(HPC)]
        KTn = [acts.tile([P, LKV], bf16, name=f"KTn{h}") for h in range(HPC)]
        V = [acts.tile([P, NCC * CL], bf16, name=f"V{t}") for t in range(2)]

        def load_weight(pool, dram, name):
            w = pool.tile([P, NCC * CL], bf16, name=name)
            for c in range(NCC):
                nc.sync.dma_start(w[:, c * CL:(c + 1) * CL],
                                  dram[c * P:(c + 1) * P, :])
            return w

        def rms_finish(ps_ss, ps, dest_ap):
            """psum ps [P, 512] holds the roped projection; rmsnorm -> dest."""
            raw = misc.tile([P, 512], f32, name="qkraw")
            nc.scalar.copy(raw[:], ps[:])
            sq = misc.tile([P, 512], bf16, name="qksq")
            nc.scalar.activation(sq[:], ps[:], FT.Square)
            ssb = ps_ss.tile([P, 512], f32, name="qkss", tag="qkss")
            nc.tensor.matmul(ssb[:], ones_b[:], sq[:], start=True, stop=True)
            rms = misc.tile([P, 512], f32, name="qkrms")
            nc.scalar.activation(rms[:], ssb[:], FT.Sqrt,
                                 bias=coef_sb[:, 4:5], scale=1.0 / D)
            inv = misc.tile([P, 512], f32, name="qkinv")
            nc.vector.reciprocal(inv[:], rms[:])
            nc.vector.tensor_mul(dest_ap, raw[:], inv[:])

        def proj_sweep(ps_proj, ps_ss, hp, t, w_sb, lb0, dests, rms):
            """One c-sweep over hT[t] cols [1024*lb0, +1024), both heads.
            dests[h] = (tile, col_off); stationary reused across the sweep."""
            banks = [[ps_proj.tile([P, 512], f32, name="pb", tag="pb")
                      for _ in range(2)] for _ in range(HPC)]
            for c in range(NCC):
                strip = hp.tile([P, 1024], bf16, name="hstrip", tag="hstrip")
                nc.sync.dma_start(
                    strip[:],
                    hT[t][c * P:(c + 1) * P, lb0 * 1024:(lb0 + 1) * 1024])
                for h in range(HPC):
                    for j in range(2):
                        nc.tensor.matmul(
                            banks[h][j][:],
                            w_sb[:, c * CL + h * D:c * CL + (h + 1) * D],
                            strip[:, j * 512:(j + 1) * 512],
                            start=(c == 0), stop=(c == NCC - 1))
            for h in range(HPC):
                for j in range(2):
                    tile_, off = dests[h]
                    ap = tile_[:, off + lb0 * 1024 + j * 512:
                               off + lb0 * 1024 + (j + 1) * 512]
                    if rms:
                        rms_finish(ps_ss, banks[h][j], ap)
                    else:
                        nc.scalar.copy(ap, banks[h][j][:])

        def kv_sweep(ps_proj, ps_ss, hp, t, wk_sb, wv_sb, q4, VT):
            """One 512-wide c-sweep computing K and V together (strip read once)."""
            kb = [ps_proj.tile([P, 512], f32, name="pb", tag="pb")
                  for _ in range(HPC)]
            vb = [ps_proj.tile([P, 512], f32, name="pb", tag="pb")
                  for _ in range(HPC)]
            for c in range(NCC):
                strip = hp.tile([P, 512], bf16, name="hstrip2", tag="hstrip2")
                nc.sync.dma_start(
                    strip[:],
                    hT[t][c * P:(c + 1) * P, q4 * 512:(q4 + 1) * 512])
                for h in range(HPC):
                    nc.tensor.matmul(
                        kb[h][:],
                        wk_sb[:, c * CL + h * D:c * CL + (h + 1) * D],
                        strip[:], start=(c == 0), stop=(c == NCC - 1))
                for h in range(HPC):
                    nc.tensor.matmul(
                        vb[h][:],
                        wv_sb[:, c * CL + h * D:c * CL + (h + 1) * D],
                        strip[:], start=(c == 0), stop=(c == NCC - 1))
            for h in range(HPC):
                rms_finish(ps_ss, kb[h],
                           KTn[h][:, t * L + q4 * 512:t * L + (q4 + 1) * 512])
                nc.scalar.copy(VT[h][:, q4 * 512:(q4 + 1) * 512], vb[h][:])

        # ================= projections =================
        with (tc.tile_pool(name="ps_proj", bufs=6, space="PSUM") as ps_proj,
              tc.tile_pool(name="hsp", bufs=10) as hp):
            with tc.tile_pool(name="wqp", bufs=1) as wqp:
                wq_sb = load_weight(wqp, wq, "wq_sb")
                with tc.tile_pool(name="ps_ss", bufs=2, space="PSUM") as ps_ss:
                    for half in range(2):
                        proj_sweep(ps_proj, ps_ss, hp, 2, wq_sb, half,
                                   [(QTn[h], 0) for h in range(HPC)], True)

            with tc.tile_pool(name="wvp", bufs=1) as wvp:
                wv_sb = load_weight(wvp, wv, "wv_sb")
                for t in range(2):
                    with tc.tile_pool(name=f"vtp{t}", bufs=1) as vtp:
                        VT = [vtp.tile([P, L], f32, name=f"VT{h}", tag=f"VT{h}")
                              for h in range(HPC)]
                        with tc.tile_pool(name=f"wk{t}p", bufs=1) as wkp:
                            wk_sb = load_weight(wkp, wk0 if t == 0 else wk1,
                                                f"wk{t}_sb")
                            with tc.tile_pool(name=f"ps_ss{t}", bufs=2,
                                              space="PSUM") as ps_ss:
                                for half in range(2):
                                    proj_sweep(ps_proj, ps_ss, hp, t, wk_sb,
                                               half,
                                               [(KTn[h], t * L) for h in range(HPC)],
                                               True)
                                    proj_sweep(ps_proj, ps_ss, hp, t, wv_sb,
                                               half,
                                               [(VT[h], 0) for h in range(HPC)],
                                               False)
                        # transpose VT -> V natural tiles (f32 in, bf16 out)
                        with tc.tile_pool(name=f"ps_tr{t}", bufs=2,
                                          space="PSUM") as ps_tr:
                            for h in range(HPC):
                                for lc in range(NCC):
                                    pt = ps_tr.tile([P, P], f32, name="pt",
                                                    tag="pt")
                                    nc.tensor.transpose(
                                        pt[:], VT[h][:, lc * P:(lc + 1) * P],
                                        ident_sb[:])
                                    nc.scalar.copy(
                                        V[t][:, lc * CL + h * D:
                                             lc * CL + (h + 1) * D], pt[:])

        misc_cm.__exit__(None, None, None)

        # ================= SIREN sinT + out-proj weights =================
        wop_cm = tc.tile_pool(name="wop", bufs=1)
        wop = wop_cm.__enter__()
        onw_sb = wop.tile([P, C], f32, name="onw_sb")
        nc.sync.dma_start(onw_sb[:], onw[:])
        wo_sb = [wop.tile([P, C], bf16, name=f"wo{j}") for j in range(2)]
        sw2_sb = [wop.tile([P, C], bf16, name=f"sw2{j}") for j in range(2)]
        for j in range(2):
            nc.sync.dma_start(wo_sb[j][:], wo[j * P:(j + 1) * P, :])
            nc.sync.dma_start(sw2_sb[j][:], sw2l[j * P:(j + 1) * P, :])
        with tc.tile_pool(name="sirp", bufs=1) as sirp:
            HW_ = L // 2
            for hf in range(2):
                ii = sirp.tile([P, HW_], i32, name="sii", tag="sii")
                nc.gpsimd.iota(ii[:], pattern=[[1, HW_]], base=hf * HW_,
                               channel_multiplier=0)
                fi = sirp.tile([P, HW_], f32, name="sfi", tag="sfi")
                nc.vector.tensor_copy(fi[:], ii[:])
                for j in range(2):
                    u = sirp.tile([P, HW_], f32, name="su", tag="su")
                    nc.vector.tensor_scalar(u[:], fi[:],
                                            coef_sb[:, j:j + 1],
                                            coef_sb[:, 2 + j:3 + j],
                                            op0=OP.mult, op1=OP.add)
                    ui = sirp.tile([P, HW_], i32, name="sui", tag="sui")
                    nc.vector.tensor_copy(ui[:], u[:])
                    uf = sirp.tile([P, HW_], f32, name="suf", tag="suf")
                    nc.vector.tensor_copy(uf[:], ui[:])
                    r = sirp.tile([P, HW_], f32, name="sr", tag="sr")
                    nc.vector.tensor_sub(r[:], u[:], uf[:])
                    nc.scalar.activation(
                        sinT[j][:, hf * HW_:(hf + 1) * HW_],
                        r[:], FT.Sin, scale=float(2 * np.pi))

        # ===== attention (q-block pairs) overlapped with out_proj + RS =====
        with (tc.tile_pool(name="expp", bufs=7) as expp,
              tc.tile_pool(name="zp", bufs=2) as zp,
              tc.tile_pool(name="opp", bufs=4) as opp,
              tc.tile_pool(name="epi", bufs=1) as epi,
              tc.tile_pool(name="ps_s", bufs=3, space="PSUM") as ps_s,
              tc.tile_pool(name="ps_o", bufs=2, space="PSUM") as ps_o,
              tc.tile_pool(name="ps_z", bufs=1, space="PSUM") as ps_z,
              tc.tile_pool(name="ps_op", bufs=2, space="PSUM") as ps_op):

            def attention_qb(qb):
                for h in range(HPC):
                    po = ps_o.tile([P, 512], f32, name="po", tag="po")
                    zacc = zp.tile([P, 512], f32, name="zacc", tag="zacc")
                    for ck in range(NCK):
                        pss = ps_s.tile([P, 512], f32, name="pss", tag="pss")
                        nc.tensor.matmul(
                            pss[:],
                            KTn[h][:, ck * P:(ck + 1) * P],
                            QTn[h][:, qb * 512:(qb + 1) * 512],
                            start=True, stop=True)
                        e = expp.tile([P, 512], bf16, name="e", tag="e")
                        nc.scalar.activation(e[:], pss[:],
                                             FT.Exp, scale=float(D ** -0.5))
                        vt, lc = ck // NCC, ck % NCC
                        nc.tensor.matmul(
                            po[:],
                            V[vt][:, lc * CL + h * D:lc * CL + (h + 1) * D],
                            e[:],
                            start=(ck == 0), stop=(ck == NCK - 1))
                        if ck == 0:
                            nc.vector.tensor_copy(zacc[:].bitcast(f32r), e[:])
                        else:
                            nc.vector.tensor_add(zacc[:].bitcast(f32r), zacc[:],
                                                 e[:])
                    pz = ps_z.tile([P, 512], f32, name="pz", tag="pz")
                    nc.tensor.matmul(pz[:], onesr, zacc[:].bitcast(f32r),
                                     start=True, stop=True)
                    invz = zp.tile([P, 512], f32, name="invz", tag="invz")
                    nc.vector.reciprocal(invz[:], pz[:])
                    nc.vector.tensor_mul(
                        OTn[h][:, qb * 512:(qb + 1) * 512], po[:], invz[:])

            def out_chunk(k):
                """out_proj rows [512k, 512k+512) + ReduceScatter + epilogue."""
                for sub in range(4):
                    lc = k * 4 + sub
                    for cb in range(4):
                        pb = ps_op.tile([P, 512], f32, name="opb", tag="opb")
                        for si, (src, rhs_sb) in enumerate(
                                [(OTn[0], wo_sb[0]), (OTn[1], wo_sb[1]),
                                 (sinT[0], sw2_sb[0]), (sinT[1], sw2_sb[1])]):
                            nc.tensor.matmul(
                                pb[:],
                                src[:, lc * P:(lc + 1) * P],
                                rhs_sb[:, cb * 512:(cb + 1) * 512],
                                start=(si == 0), stop=(si == 3))
                        t_ = opp.tile([P, 512], f32, name="opt", tag="opt")
                        nc.scalar.copy(t_[:], pb[:])
                        nc.sync.dma_start(
                            partial[k][sub * P:(sub + 1) * P,
                                       cb * 512:(cb + 1) * 512],
                            t_[:])
                nc.gpsimd.collective_compute(
                    "ReduceScatter", OP.add,
                    replica_groups=[list(range(NCORES))],
                    ins=[partial[k][:]],
                    outs=[rs_out[k][:]],
                )

            def epilogue_chunk(k):
                sh = epi.tile([SH, C], f32, name="sh", tag="sh")
                nc.sync.dma_start(sh[:], rs_out[k][:])
                scr = epi.tile([SH, C], f32, name="scr", tag="scr")
                ssq = epi.tile([SH, 1], f32, name="ssq", tag="ssq")
                nc.scalar.activation(scr[:], sh[:], FT.Square, accum_out=ssq[:])
                rmst = epi.tile([SH, 1], f32, name="rmst", tag="rmst")
                nc.scalar.activation(rmst[:], ssq[:], FT.Sqrt,
                                     bias=coef_sb[:SH, 4:5], scale=1.0 / C)
                rinv = epi.tile([SH, 1], f32, name="rinv", tag="rinv")
                nc.vector.reciprocal(rinv[:], rmst[:])
                xt = epi.tile([SH, C], f32, name="xt", tag="xt")
                nc.sync.dma_start(xt[:], xs[k * SH:(k + 1) * SH, :])
                nc.vector.scalar_tensor_tensor(
                    scr[:], sh[:], rinv[:], onw_sb[:SH, :],
                    op0=OP.mult, op1=OP.mult)
                nc.vector.tensor_add(scr[:], scr[:], xt[:])
                nc.sync.dma_start(out[k * SH:(k + 1) * SH, :], scr[:])

            for qb in range(NQB):
                attention_qb(qb)
                out_chunk(qb)
            for k in range(NQB):
                epilogue_chunk(k)

        wop_cm.__exit__(None, None, None)
        acts_cm.__exit__(None, None, None)

    nc.compile()
    return nc


def _rope_mat(depth: float) -> np.ndarray:
    half = D // 2
    freqs = 1.0 / 10000.0 ** (np.arange(half, dtype=np.float32) / half)
    ang = np.float32(depth) * freqs
    c, s = np.cos(ang).astype(np.float32), np.sin(ang).astype(np.float32)
    R = np.zeros((D, D), np.float32)
    R[np.arange(half), np.arange(half)] = c
    R[np.arange(half), np.arange(half) + half] = -s
    R[np.arange(half) + half, np.arange(half)] = s
    R[np.arange(half) + half, np.arange(half) + half] = c
    return R


def _fold_weights(W, norm_w, depth):
    """Per head: R_depth @ diag(norm_w) @ W_head  (rope and norm weight folded)."""
    R = _rope_mat(depth)
    out = np.empty_like(W)
    nheads = W.shape[0] // D
    for h in range(nheads):
        out[h * D:(h + 1) * D] = R @ (norm_w[:, None] * W[h * D:(h + 1) * D])
    return out


def kernel(**inputs) -> np.ndarray:
    inputs = {k: np.asarray(v, dtype=np.float32) if np.asarray(v).dtype != np.int32
              else np.asarray(v) for k, v in inputs.items()}
    x = inputs["x"]
    qn, kn = inputs["qn_w"], inputs["kn_w"]

    # rmsnorm scale is computed on-device from the roped/weighted projection;
    # exact when qn_w/kn_w are all ones (rope is orthogonal).
    if not (np.allclose(qn, 1.0) and np.allclose(kn, 1.0)):
        raise NotImplementedError("non-unit q/k norm weights not supported")

    if "prog" not in _CACHE:
        _CACHE["prog"] = _build_program()
    nc = _CACHE["prog"]

    hTb = [np.ascontiguousarray(inputs[f"h{t}"][0].T).astype(BF)
           for t in range(3)]
    sb2 = inputs["sb2"]
    assert not np.any(sb2), "nonzero sb2 not folded in"  # setup uses zeros

    in_maps = []
    for i in range(NCORES):
        sl = slice(i * CL, (i + 1) * CL)
        wq_f = _fold_weights(inputs["Wq"][sl], qn, 2.0)
        wk0_f = _fold_weights(inputs["Wk"][sl], kn, 0.0)
        wk1_f = _fold_weights(inputs["Wk"][sl], kn, 1.0)
        a = (2.0 * 30.0 * inputs["sw1"][0, sl] / (L - 1)).astype(np.float32)
        b = (30.0 * (inputs["sb1"][sl] - inputs["sw1"][0, sl])).astype(np.float32)
        coef = np.zeros((P, 6), np.float32)
        coef[:, 4] = EPS
        coef[:, 0], coef[:, 1] = a[:P], a[P:]
        coef[:, 2], coef[:, 3] = b[:P], b[P:]
        inv2pi = np.float32(1.0 / (2 * np.pi))
        coef[:, :2] *= inv2pi
        coef[:, 2:4] *= inv2pi
        xsl = np.concatenate([x[0, k * 512 + i * SH:k * 512 + (i + 1) * SH, :]
                              for k in range(NQB)], axis=0)
        in_maps.append({
            "hT0": hTb[0], "hT1": hTb[1], "hT2": hTb[2],
            "wq": np.ascontiguousarray(wq_f.T).astype(BF),
            "wk0": np.ascontiguousarray(wk0_f.T).astype(BF),
            "wk1": np.ascontiguousarray(wk1_f.T).astype(BF),
            "wv": np.ascontiguousarray(inputs["Wv"][sl].T).astype(BF),
            "wo": np.ascontiguousarray(inputs["Wo"][:, sl].T).astype(BF),
            "sw2l": np.ascontiguousarray(inputs["sw2"][sl, :]).astype(BF),
            "coef": coef,
            "ident": np.eye(P, dtype=np.float32),
            "onw": np.ascontiguousarray(
                np.broadcast_to(inputs["on_w"][None, :], (P, C))),
            "xs": np.ascontiguousarray(xsl),
        })

    _CACHE["last_in_maps"] = in_maps
    res = run_bass_kernel_spmd(nc, in_maps, list(range(NCORES)))
    out = np.empty((1, L, C), np.float32)
    for i in range(NCORES):
        o = res.results[i]["o"]
        for k in range(NQB):
            out[0, k * 512 + i * SH:k * 512 + (i + 1) * SH, :] = \
                o[k * SH:(k + 1) * SH, :]
    return out



# revision 5
# speedup vs baseline: 1.6726x; 1.6726x over previous
"""TRN2 Bass kernel for nn_CrossLayerAttention: head-parallel tensor-parallel
over 8 NeuronCores.

v2: fp8 DoubleRow matmuls on every softmax-suppressed path, SIREN folded on
host, batched activations, PE-order-aware emission.

Per core i (2 heads, local channel slice sl = [256i, 256i+256)):
  - hT = h.T cast to fp8 e4m3, streamed from DRAM in [128, 2, 512] pair strips
  - Q/K projections: fp8 DoubleRow (256-deep contraction per instruction) with
    rope+scale folded into fp8 weights (x32; scale cancels in the on-device
    rmsnorm). rms = rsqrt(mean(x^2)+eps) via Square -> ones-matmul -> Rsqrt.
  - V projected directly in natural [kv, d] layout (strip chunks as stationary,
    fp8 weights x16 as moving; scale cancels in the final rmsnorm), stored as
    fp8 DoubleRow pairs. No PE transposes.
  - attention: QK in bf16 (q/k normalized, ST layout), exp over [128,1024]
    PSUM pair -> fp8 e pairs; AV fp8 DoubleRow; Z accumulated bf16 on DVE +
    ones-matmul; OT = po/Z in bf16.
  - out_proj bf16 (direct output-noise path stays 16-bit); SIREN positional
    field computed on host (input-independent) and added in the epilogue.
  - partials in bf16, chunked ReduceScatter overlapped with later attention
    blocks; a dummy warmup collective at program start absorbs ring setup.
"""
import numpy as np
import ml_dtypes
from contextlib import ExitStack

import concourse.bass as bass
import concourse.tile as tile
from concourse import bacc, mybir
from concourse.bass_utils import run_bass_kernel_spmd

P = 128
L = 2048
C = 2048
H = 16
D = 128
NCORES = 8
HPC = H // NCORES          # heads per core
CL = HPC * D               # local channels per core
LKV = 2 * L                # kv length (2 history entries)
EPS = 1e-6
NQB = L // 512             # q blocks / RS chunks (4)
SH = L // 8 // NQB         # shard rows per RS chunk (64)
WS = 32.0                  # wq/wk fp8 scale (cancels in q/k rmsnorm)
VS = 16.0                  # wv fp8 scale (cancels in final rmsnorm)

f32 = mybir.dt.float32
bf16 = mybir.dt.bfloat16
f8 = mybir.dt.float8e4
FT = mybir.ActivationFunctionType
OP = mybir.AluOpType
PM = mybir.MatmulPerfMode
BF = ml_dtypes.bfloat16
F8 = ml_dtypes.float8_e4m3

_CACHE = {}


def _build_program():
    nc = bacc.Bacc("TRN2", target_bir_lowering=False, debug=False,
                   num_devices=NCORES)

    # ---- DRAM I/O ----
    hT = [nc.dram_tensor(f"hT{t}", [C, L], f8, kind="ExternalInput")
          for t in range(3)]
    wq = nc.dram_tensor("wq", [P, 4096], f8, kind="ExternalInput")
    wk0 = nc.dram_tensor("wk0", [P, 4096], f8, kind="ExternalInput")
    wk1 = nc.dram_tensor("wk1", [P, 4096], f8, kind="ExternalInput")
    wv = nc.dram_tensor("wv", [P, 4096], f8, kind="ExternalInput")
    wo = nc.dram_tensor("wo", [CL, C], bf16, kind="ExternalInput")
    onw = nc.dram_tensor("onw", [P, C], f32, kind="ExternalInput")
    xs = nc.dram_tensor("xs", [NQB * SH, C], f32, kind="ExternalInput")
    pos = nc.dram_tensor("pos", [NQB * SH, C], bf16, kind="ExternalInput")
    out = nc.dram_tensor("o", [NQB * SH, C], f32, kind="ExternalOutput")

    partial = [nc.dram_tensor(f"partial{k}", [512, C], bf16)
               for k in range(NQB)]
    rs_out = [nc.dram_tensor(f"rs_out{k}", [SH, C], bf16) for k in range(NQB)]
    wdum = nc.dram_tensor("wdum", [8, 8], f32)
    wrs = nc.dram_tensor("wrs", [1, 8], f32)

    with tile.TileContext(nc) as tc, ExitStack() as ctx:
        const = ctx.enter_context(tc.tile_pool(name="const", bufs=1))
        persist = ctx.enter_context(tc.tile_pool(name="persist", bufs=1))

        # ---- constants ----
        ones_t = const.tile([P, P], f32)
        nc.vector.memset(ones_t[:], 1.0)
        ones_b = const.tile([P, P], bf16)
        nc.vector.tensor_copy(ones_b[:], ones_t[:])
        eps_t = const.tile([P, 1], f32)
        nc.vector.memset(eps_t[:], EPS)

        # ---- warmup collective (absorb ring setup during projections) ----
        wdum_sb = const.tile([8, 8], f32)
        nc.vector.memset(wdum_sb[:], 0.0)
        nc.sync.dma_start(wdum[:], wdum_sb[:])
        nc.gpsimd.collective_compute(
            "ReduceScatter", OP.add,
            replica_groups=[list(range(NCORES))],
            ins=[wdum[:]], outs=[wrs[:]],
        )

        # ---- persistent activations / weights ----
        QTa = persist.tile([P, HPC, L], bf16, name="QTa")
        KTa = persist.tile([P, HPC, LKV], bf16, name="KTa")
        Va = persist.tile([P, LKV // 256, 2, CL], f8, name="Va")
        OTa = persist.tile([P, HPC, L], bf16, name="OTa")
        wo_sb = persist.tile([P, HPC, C], bf16, name="wo_sb")
        onw_sb = persist.tile([P, C], f32, name="onw_sb")
        nc.sync.dma_start(onw_sb[:], onw[:])
        for h in range(HPC):
            nc.sync.dma_start(wo_sb[:, h, :], wo[h * P:(h + 1) * P, :])

        # ================= projections =================
        wp_cm = tc.tile_pool(name="wp", bufs=1)
        wp = wp_cm.__enter__()
        wq_sb = wp.tile([P, 8, 2, HPC, D], f8, name="wq_sb")
        wk_sb = [wp.tile([P, 8, 2, HPC, D], f8, name=f"wk{t}_sb")
                 for t in range(2)]
        wv_sb = wp.tile([P, 8, 2, CL], f8, name="wv_sb")
        nc.sync.dma_start(wq_sb[:].rearrange("p a j h m -> p (a j h m)"), wq[:])
        nc.sync.dma_start(
            wk_sb[0][:].rearrange("p a j h m -> p (a j h m)"), wk0[:])
        nc.sync.dma_start(
            wk_sb[1][:].rearrange("p a j h m -> p (a j h m)"), wk1[:])
        nc.sync.dma_start(wv_sb[:].rearrange("p a j m -> p (a j m)"), wv[:])

        pp_cm = tc.tile_pool(name="pp", bufs=1, space="PSUM")
        pp = pp_cm.__enter__()
        hp_cm = tc.tile_pool(name="hp", bufs=6)
        hp = hp_cm.__enter__()
        rp_cm = tc.tile_pool(name="rp", bufs=2)
        rp = rp_cm.__enter__()

        def sweep(t, q4, w_sb, dst_tile, dst_off, with_v):
            """Project hT[t] cols [512*q4, +512): K (and V) for both heads."""
            kps = pp.tile([P, 1024], f32, name="kps", tag="kps")
            if with_v:
                vps = [pp.tile([P, 256], f32, name=f"vb{lb}", tag=f"vb{lb}")
                       for lb in range(4)]
            for cc in range(8):
                strip = hp.tile([P, 2, 512], f8, name="strip", tag="strip")
                sf = strip[:].rearrange("p j q -> p (j q)")
                for j in range(2):
                    nc.sync.dma_start(
                        sf[:, j * 512:(j + 1) * 512],
                        hT[t][cc * 256 + j * P:cc * 256 + (j + 1) * P,
                              q4 * 512:(q4 + 1) * 512])
                for h in range(HPC):
                    nc.tensor.matmul(
                        kps[:, h * 512:(h + 1) * 512],
                        w_sb[:, cc, :, h, :], strip[:],
                        start=(cc == 0), stop=(cc == 7),
                        perf_mode=PM.DoubleRow)
                if with_v:
                    for lb in range(4):
                        nc.tensor.matmul(
                            vps[lb][:],
                            strip[:, :, lb * P:(lb + 1) * P],
                            wv_sb[:, cc, :, :],
                            start=(cc == 0), stop=(cc == 7),
                            perf_mode=PM.DoubleRow)
            if with_v:
                for lb in range(4):
                    ck = t * 16 + q4 * 4 + lb
                    nc.scalar.copy(Va[:, ck // 2, ck % 2, :], vps[lb][:])
            # rmsnorm: drain psum early (bf16 raw), then scale by rsqrt(ms+eps)
            raw = rp.tile([P, 1024], bf16, name="raw", tag="raw")
            nc.vector.tensor_copy(raw[:], kps[:])
            sq = rp.tile([P, 1024], bf16, name="sq", tag="sq")
            nc.scalar.activation(sq[:], raw[:], FT.Square)
            ssq = pp.tile([P, 1024], f32, name="ssq", tag="ssq")
            for half in range(2):
                nc.tensor.matmul(ssq[:, half * 512:(half + 1) * 512],
                                 ones_b[:], sq[:, half * 512:(half + 1) * 512],
                                 start=True, stop=True)
            rms = rp.tile([P, 1024], f32, name="rms", tag="rms")
            nc.scalar.activation(rms[:], ssq[:], FT.Sqrt,
                                 bias=eps_t[:, 0:1], scale=1.0 / D)
            inv = rp.tile([P, 1024], f32, name="inv", tag="inv")
            scr8 = rp.tile([P, 1024], f32, name="scr8", tag="scr8")
            nc.vector.reciprocal_approx_accurate(inv[:], rms[:], scr8[:])
            nc.vector.tensor_mul(
                dst_tile[:, :, dst_off:dst_off + 512],
                raw[:].rearrange("p (h q) -> p h q", h=2),
                inv[:].rearrange("p (h q) -> p h q", h=2))

        for t in range(2):
            for q4 in range(4):
                sweep(t, q4, wk_sb[t], KTa, t * L + q4 * 512, True)
        for q4 in range(4):
            sweep(2, q4, wq_sb, QTa, q4 * 512, False)

        rp_cm.__exit__(None, None, None)
        hp_cm.__exit__(None, None, None)
        pp_cm.__exit__(None, None, None)
        wp_cm.__exit__(None, None, None)

        # ===== attention / out_proj / RS / epilogue, PE-order interleaved ====
        with (tc.tile_pool(name="pssp", bufs=2, space="PSUM") as pssp,
              tc.tile_pool(name="pozp", bufs=2, space="PSUM") as pozp,
              tc.tile_pool(name="pbp", bufs=2, space="PSUM") as pbp,
              tc.tile_pool(name="ep", bufs=4) as ep,
              tc.tile_pool(name="zp", bufs=2) as zp,
              tc.tile_pool(name="ob", bufs=3) as ob,
              tc.tile_pool(name="epi", bufs=1) as epi):

            def att(qb, h):
                po = pozp.tile([P, 512], f32, name="po", tag="poz")
                zacc = zp.tile([P, 1024], bf16, name="zacc", tag="zacc")
                q_ap = QTa[:, h, qb * 512:(qb + 1) * 512]
                pend = None  # delayed AV emission keeps PE queue stall-free
                for cc in range(16):
                    pss = pssp.tile([P, 1024], f32, name="pss", tag="pss")
                    for j in range(2):
                        ck = 2 * cc + j
                        nc.tensor.matmul(
                            pss[:, j * 512:(j + 1) * 512],
                            KTa[:, h, ck * P:(ck + 1) * P], q_ap,
                            start=True, stop=True)
                    if pend is not None:
                        nc.tensor.matmul(
                            po[:], Va[:, cc - 1, :, h * D:(h + 1) * D],
                            pend[:], start=(cc == 1), stop=False,
                            perf_mode=PM.DoubleRow)
                    e = ep.tile([P, 2, 512], f8, name="e", tag="e")
                    ef = e[:].rearrange("p j q -> p (j q)")
                    nc.scalar.activation(ef, pss[:], FT.Exp,
                                         scale=float(D ** -0.5))
                    if cc == 0:
                        nc.vector.tensor_copy(zacc[:], ef)
                    else:
                        nc.vector.tensor_add(zacc[:], zacc[:], ef)
                    pend = e
                nc.tensor.matmul(po[:], Va[:, 15, :, h * D:(h + 1) * D],
                                 pend[:], start=False, stop=True,
                                 perf_mode=PM.DoubleRow)
                pz = pozp.tile([P, 512], f32, name="pz", tag="poz")
                for half in range(2):
                    nc.tensor.matmul(pz[:], ones_b[:],
                                     zacc[:, half * 512:(half + 1) * 512],
                                     start=(half == 0), stop=(half == 1))
                invz = zp.tile([P, 512], f32, name="invz", tag="invz")
                zscr = zp.tile([P, 512], f32, name="zscr", tag="zscr")
                nc.vector.reciprocal_approx_accurate(invz[:], pz[:], zscr[:])
                nc.vector.tensor_mul(OTa[:, h, qb * 512:(qb + 1) * 512],
                                     po[:], invz[:])

            def out_chunk(k):
                """out_proj rows [512k, +512) in bf16 + chunked ReduceScatter."""
                for sub in range(4):
                    lc = k * 4 + sub
                    for cb in range(4):
                        pb = pbp.tile([P, 512], f32, name="pb", tag="pb")
                        for h in range(HPC):
                            nc.tensor.matmul(
                                pb[:], OTa[:, h, lc * P:(lc + 1) * P],
                                wo_sb[:, h, cb * 512:(cb + 1) * 512],
                                start=(h == 0), stop=(h == HPC - 1))
                        tb = ob.tile([P, 512], bf16, name="tb", tag="tb")
                        nc.scalar.copy(tb[:], pb[:])
                        nc.sync.dma_start(
                            partial[k][sub * P:(sub + 1) * P,
                                       cb * 512:(cb + 1) * 512], tb[:])
                nc.gpsimd.collective_compute(
                    "ReduceScatter", OP.add,
                    replica_groups=[list(range(NCORES))],
                    ins=[partial[k][:]], outs=[rs_out[k][:]],
                )

            def epilogue(k):
                shb = epi.tile([SH, C], bf16, name="shb", tag="shb")
                nc.sync.dma_start(shb[:], rs_out[k][:])
                posb = epi.tile([SH, C], bf16, name="posb", tag="posb")
                nc.sync.dma_start(posb[:], pos[k * SH:(k + 1) * SH, :])
                shf = epi.tile([SH, C], f32, name="shf", tag="shf")
                nc.vector.tensor_add(shf[:], shb[:], posb[:])
                scr = epi.tile([SH, C], f32, name="scr", tag="scr")
                ssqt = epi.tile([SH, 1], f32, name="ssqt", tag="ssqt")
                nc.scalar.activation(scr[:], shf[:], FT.Square,
                                     accum_out=ssqt[:])
                rmst = epi.tile([SH, 1], f32, name="rmst", tag="rmst")
                nc.scalar.activation(rmst[:], ssqt[:], FT.Sqrt,
                                     bias=eps_t[:SH, 0:1], scale=1.0 / C)
                rinv = epi.tile([SH, 1], f32, name="rinv", tag="rinv")
                nc.vector.reciprocal(rinv[:], rmst[:])
                xt = epi.tile([SH, C], f32, name="xt", tag="xt")
                nc.sync.dma_start(xt[:], xs[k * SH:(k + 1) * SH, :])
                nc.vector.scalar_tensor_tensor(
                    scr[:], shf[:], rinv[:], onw_sb[:SH, :],
                    op0=OP.mult, op1=OP.mult)
                nc.vector.tensor_add(scr[:], scr[:], xt[:])
                nc.sync.dma_start(out[k * SH:(k + 1) * SH, :], scr[:])

            # emission order keeps the in-order PE queue busy across the
            # invz/OTa tail of each head and overlaps RS + epilogue
            att(0, 0)
            att(0, 1)
            att(1, 0)
            out_chunk(0)
            att(1, 1)
            att(2, 0)
            out_chunk(1)
            epilogue(0)
            att(2, 1)
            att(3, 0)
            out_chunk(2)
            epilogue(1)
            att(3, 1)
            out_chunk(3)
            epilogue(2)
            epilogue(3)

    nc.compile()
    return nc


def _rope_mat(depth: float) -> np.ndarray:
    half = D // 2
    freqs = 1.0 / 10000.0 ** (np.arange(half, dtype=np.float32) / half)
    ang = np.float32(depth) * freqs
    c, s = np.cos(ang).astype(np.float32), np.sin(ang).astype(np.float32)
    R = np.zeros((D, D), np.float32)
    R[np.arange(half), np.arange(half)] = c
    R[np.arange(half), np.arange(half) + half] = -s
    R[np.arange(half) + half, np.arange(half)] = s
    R[np.arange(half) + half, np.arange(half) + half] = c
    return R


def _fold_weights(W, norm_w, depth):
    """Per head: R_depth @ diag(norm_w) @ W_head  (rope and norm weight folded)."""
    R = _rope_mat(depth)
    out = np.empty_like(W)
    nheads = W.shape[0] // D
    for h in range(nheads):
        out[h * D:(h + 1) * D] = R @ (norm_w[:, None] * W[h * D:(h + 1) * D])
    return out


def _pack_qk(wf):
    """[CL, C] stationary -> [128, (cc j h m)] fp8 DoubleRow layout."""
    wt = np.ascontiguousarray(wf.T)              # [C, CL]
    wt = wt.reshape(8, 2, P, HPC, D).transpose(2, 0, 1, 3, 4)
    return np.ascontiguousarray(wt.reshape(P, 4096)).astype(F8)


def _pack_v(wf):
    """[CL, C] moving -> [128, (cc j m)] fp8 DoubleRow layout."""
    wt = np.ascontiguousarray(wf.T)              # [C, CL]
    wt = wt.reshape(8, 2, P, CL).transpose(2, 0, 1, 3)
    return np.ascontiguousarray(wt.reshape(P, 4096)).astype(F8)


def kernel(**inputs) -> np.ndarray:
    inputs = {k: np.asarray(v, dtype=np.float32) if np.asarray(v).dtype != np.int32
              else np.asarray(v) for k, v in inputs.items()}
    x = inputs["x"]
    qn, kn = inputs["qn_w"], inputs["kn_w"]

    # rmsnorm scale is computed on-device from the roped/weighted projection;
    # exact when qn_w/kn_w are all ones (rope is orthogonal).
    if not (np.allclose(qn, 1.0) and np.allclose(kn, 1.0)):
        raise NotImplementedError("non-unit q/k norm weights not supported")

    if "prog" not in _CACHE:
        _CACHE["prog"] = _build_program()
    nc = _CACHE["prog"]

    hT8 = [np.ascontiguousarray(inputs[f"h{t}"][0].T).astype(F8)
           for t in range(3)]

    # SIREN positional field is input-independent: fold on host (x VS to match
    # the on-device scale; the final rmsnorm cancels it).
    coords = np.linspace(-1.0, 1.0, L, dtype=np.float32)[:, None]
    posf = (np.sin(30.0 * (coords @ inputs["sw1"] + inputs["sb1"][None, :]))
            @ inputs["sw2"] + inputs["sb2"][None, :]) * np.float32(VS)

    in_maps = []
    for i in range(NCORES):
        sl = slice(i * CL, (i + 1) * CL)
        wq_f = _fold_weights(inputs["Wq"][sl], qn, 2.0) * np.float32(WS)
        wk0_f = _fold_weights(inputs["Wk"][sl], kn, 0.0) * np.float32(WS)
        wk1_f = _fold_weights(inputs["Wk"][sl], kn, 1.0) * np.float32(WS)
        wv_f = inputs["Wv"][sl] * np.float32(VS)
        rows = np.concatenate(
            [np.arange(k * 512 + i * SH, k * 512 + (i + 1) * SH)
             for k in range(NQB)])
        in_maps.append({
            "hT0": hT8[0], "hT1": hT8[1], "hT2": hT8[2],
            "wq": _pack_qk(wq_f),
            "wk0": _pack_qk(wk0_f),
            "wk1": _pack_qk(wk1_f),
            "wv": _pack_v(wv_f),
            "wo": np.ascontiguousarray(inputs["Wo"][:, sl].T).astype(BF),
            "onw": np.ascontiguousarray(
                np.broadcast_to(inputs["on_w"][None, :], (P, C))),
            "xs": np.ascontiguousarray(x[0, rows, :]),
            "pos": np.ascontiguousarray(posf[rows, :]).astype(BF),
        })

    _CACHE["last_in_maps"] = in_maps
    res = run_bass_kernel_spmd(nc, in_maps, list(range(NCORES)))
    out = np.empty((1, L, C), np.float32)
    for i in range(NCORES):
        o = res.results[i]["o"]
        for k in range(NQB):
            out[0, k * 512 + i * SH:k * 512 + (i + 1) * SH, :] = \
                o[k * SH:(k + 1) * SH, :]
    return out
